# revision 4
# baseline (speedup 1.0000x reference)
"""MoE BaseRouter kernel for Trainium2 (8 NeuronCores, SPMD over tokens).

Computes, for h [T=16384, D=4096] f32, W [D, E=128] f32, token_mask [T] bool:
  logits_clean = h @ W
  logits_sel   = logits_clean + gumbel          (tau=1, temps=1)
  top-8 expert mask over logits_sel (per token)
  probs        = renormalized softmax(logits_clean) over the selected experts
returning (mask_full bool, probs f32, logits_clean f32, logits_sel f32),
mirroring the reference BaseRouter.

Strategy: tokens sharded 2048/core across 8 cores; W replicated. h is
transposed on the host (hT, [D, Tshard]) so the GEMM contraction dim D sits
on SBUF partitions. Per core: 4 groups of 512 tokens; each group accumulates
32 fp32 matmuls (W chunk stationary [128d,128E], hT chunk moving [128d,512t])
into PSUM [E,512], PE-transposes 128x128 blocks to token-major, then a short
DVE/ACT stage does top-8 (single InstMax), threshold mask, exp and masked
renormalization. Gumbel noise is a fixed PRNG constant (jax threefry,
key fold_in(key(7),1)) computed on the host CPU and streamed in.
"""

import numpy as np

T, D, E, K = 16384, 4096, 128, 8
NCORES = 8
TSH = T // NCORES  # tokens per core (2048)
NG = 4  # token groups per core
GT = TSH // NG  # tokens per group (512)
NB = GT // 128  # 128-token blocks per group (4)
NK = D // 128  # contraction chunks (32)

_cache = {}


def _apply_tile_patch(tile_mod, mybir):
    """walrus CoreV3 codegen allows at most 1 sync wait per instruction; the
    TileContext tail drain carries one wait per live proc. Spill them onto
    SP nops (order only needs every wait to precede the semaphore clear)."""
    if getattr(tile_mod.TileContext, "_drain_patched", False):
        return

    def _drain_and_barrier(self, tick_clock, wait_clock):
        nc = self.nc
        drain_inst = nc.sync.drain()
        wait_clock.add_sem_waits(
            drain_inst.ins, tile_mod.ScopedClock({None: tick_clock.global_clock})
        )
        si = drain_inst.ins.sync_info
        if si is not None and si.on_wait:
            waits = list(si.on_wait)
            del si.on_wait[:]
            for w in waits:
                nop = nc.sync.nop(nofuse=True, hint="drain_wait_spill")
                nop.ins.sync_info = mybir.SyncInfo(on_update=[], on_wait=[w])
        nc.all_engine_barrier()
        assert self.sems is not None
        popped = nc._tile_sem_poison_stack.pop()
        assert popped is self._sem_poison
        nc.clear_and_free_semaphores(list(self.sems.allocated().values()))
        nc.all_engine_barrier()

    tile_mod.TileContext._drain_and_barrier = _drain_and_barrier
    tile_mod.TileContext._drain_patched = True


def _build_nc():
    """Build the per-core Bass module (same program on all 8 cores)."""
    from contextlib import ExitStack

    import concourse.tile as tile
    from concourse import bacc, mybir

    f32 = mybir.dt.float32
    u8 = mybir.dt.uint8

    nc = bacc.Bacc(
        "TRN2", target_bir_lowering=False, debug=False, enable_asserts=False
    )
    hT = nc.dram_tensor("hT", [D, TSH], f32, kind="ExternalInput")
    Wm = nc.dram_tensor("Wm", [D, E], f32, kind="ExternalInput")
    gum = nc.dram_tensor("gum", [TSH, E], f32, kind="ExternalInput")
    ident = nc.dram_tensor("ident", [128, 128], f32, kind="ExternalInput")
    lclean = nc.dram_tensor("lclean", [TSH, E], f32, kind="ExternalOutput")
    lsel = nc.dram_tensor("lsel", [TSH, E], f32, kind="ExternalOutput")
    probs = nc.dram_tensor("probs", [TSH, E], f32, kind="ExternalOutput")
    mask = nc.dram_tensor("mask", [TSH, E], u8, kind="ExternalOutput")

    # DRAM views: d = 128*k + p ; token = 512*g + 128*b + p
    hT_v = hT[:, :].rearrange("(k p) j -> p k j", p=128)  # [128, NK, TSH]
    W_v = Wm[:, :].rearrange("(k p) e -> p k e", p=128)  # [128, NK, E]

    def tok_view(t):  # [TSH, E] -> [NG][128, NB, E]
        return t[:, :].rearrange("(g b p) e -> g p b e", g=NG, b=NB)

    lclean_v, lsel_v, probs_v, mask_v, gum_v = (
        tok_view(x) for x in (lclean, lsel, probs, mask, gum)
    )

    with tile.TileContext(nc) as tc, ExitStack() as ctx:
        wpool = ctx.enter_context(tc.tile_pool(name="w", bufs=1))
        cpool = ctx.enter_context(tc.tile_pool(name="const", bufs=1))
        hpool = ctx.enter_context(tc.tile_pool(name="h", bufs=8))
        ppool = ctx.enter_context(tc.tile_pool(name="psum_mm", bufs=2, space="PSUM"))
        ptpool = ctx.enter_context(tc.tile_pool(name="psum_t", bufs=6, space="PSUM"))
        lgpool = ctx.enter_context(tc.tile_pool(name="lg", bufs=2))
        opool = ctx.enter_context(tc.tile_pool(name="outs", bufs=2))
        spool = ctx.enter_context(tc.tile_pool(name="small", bufs=16))

        W_sb = wpool.tile([128, NK, E], f32)
        nc.sync.dma_start(W_sb[:], W_v)
        id_sb = cpool.tile([128, 128], f32)
        nc.sync.dma_start(id_sb[:], ident[:, :])

        for g in range(NG):
            # ---- load hT group slab in 4 sub-DMAs of 8 chunks each ----
            hs = []
            for q in range(4):
                h_sb = hpool.tile([128, 8, GT], f32, tag="hslab")
                nc.sync.dma_start(
                    h_sb[:], hT_v[:, 8 * q : 8 * q + 8, g * GT : (g + 1) * GT]
                )
                hs.append(h_sb)

            # ---- GEMM: psum[E, GT] += W[:,k,:].T @ hT[:,k,:] over 32 chunks
            psum_g = ppool.tile([E, GT], f32, tag="psg")
            for k in range(NK):
                nc.tensor.matmul(
                    psum_g[:],
                    W_sb[:, k, :],
                    hs[k // 8][:, k % 8, :],
                    start=(k == 0),
                    stop=(k == NK - 1),
                )

            # ---- PSUM -> SBUF, then PE-transpose to token-major blocks ----
            lg_sb = lgpool.tile([E, GT], f32, tag="lg")
            nc.scalar.copy(lg_sb[:], psum_g[:])

            gum_sb = opool.tile([128, NB, E], f32, tag="gum")
            nc.sync.dma_start(gum_sb[:], gum_v[g])
            lclean_sb = opool.tile([128, NB, E], f32, tag="lclean")
            lsel_sb = opool.tile([128, NB, E], f32, tag="lsel")
            probs_sb = opool.tile([128, NB, E], f32, tag="probs")
            mask_sb = opool.tile([128, NB, E], u8, tag="mask")

            for b in range(NB):
                lt = ptpool.tile([128, E], f32, tag="pst")  # logits [tok, E] in PSUM
                nc.tensor.transpose(lt[:], lg_sb[:, 128 * b : 128 * (b + 1)], id_sb[:])

                # logits_sel = logits + gumbel (also the lsel output tile)
                xs = lsel_sb[:, b, :]
                nc.vector.tensor_tensor(xs, lt[:], gum_sb[:, b, :], mybir.AluOpType.add)

                # top-8 values; threshold = 8th largest
                m8 = spool.tile([128, 8], f32, tag="m8")
                nc.vector.max(m8[:], xs)
                thr = m8[:, 7:8]
                maskf = spool.tile([128, E], f32, tag="maskf")
                nc.vector.tensor_scalar(maskf[:], xs, thr, None, mybir.AluOpType.is_ge)
                nc.vector.tensor_scalar(
                    mask_sb[:, b, :], xs, thr, None, mybir.AluOpType.is_ge
                )

                # renormalized softmax over selected experts
                nmax = spool.tile([128, 1], f32, tag="nmax")
                nc.vector.tensor_reduce(
                    nmax[:], lt[:], mybir.AxisListType.X, mybir.AluOpType.max,
                    negate=True,
                )
                et = spool.tile([128, E], f32, tag="et")
                nc.scalar.activation(
                    et[:], lt[:], mybir.ActivationFunctionType.Exp,
                    bias=nmax[:, 0:1], scale=1.0,
                )
                nc.scalar.copy(lclean_sb[:, b, :], lt[:])
                pb = probs_sb[:, b, :]
                nc.vector.tensor_tensor(pb, et[:], maskf[:], mybir.AluOpType.mult)
                ssum = spool.tile([128, 1], f32, tag="ssum")
                nc.vector.reduce_sum(ssum[:], pb, axis=mybir.AxisListType.X)
                rec = spool.tile([128, 1], f32, tag="rec")
                nc.vector.reciprocal(rec[:], ssum[:])
                nc.vector.tensor_scalar_mul(pb, pb, rec[:, 0:1])

            nc.scalar.dma_start(lclean_v[g], lclean_sb[:])
            nc.scalar.dma_start(lsel_v[g], lsel_sb[:])
            nc.scalar.dma_start(probs_v[g], probs_sb[:])
            nc.scalar.dma_start(mask_v[g], mask_sb[:])

    nc.compile()
    return nc


def _gumbel_np():
    """The reference's gumbel draw — a fixed constant (jax threefry on CPU)."""
    import jax
    import jax.numpy as jnp

    cpu = jax.devices("cpu")[0]
    with jax.default_device(cpu):
        kg = jax.random.fold_in(jax.random.key(7), 1)
        u = jax.random.uniform(
            kg, (T, E), minval=1e-06, maxval=1 - 1e-06, dtype=jnp.float32
        )
        g = -jnp.log(-jnp.log(u))
        return np.asarray(jax.device_get(g), dtype=np.float32)


def _run_device(h, W):
    from concourse.bass_utils import run_bass_kernel_spmd

    if "nc" not in _cache:
        _cache["nc"] = _build_nc()
    if "gum" not in _cache:
        _cache["gum"] = _gumbel_np()
    gum = _cache["gum"]
    ident = np.eye(128, dtype=np.float32)
    Wc = np.ascontiguousarray(W, dtype=np.float32)

    in_maps = []
    for c in range(NCORES):
        sl = slice(c * TSH, (c + 1) * TSH)
        in_maps.append(
            {
                "hT": np.ascontiguousarray(h[sl].T),
                "Wm": Wc,
                "gum": np.ascontiguousarray(gum[sl]),
                "ident": ident,
            }
        )
    res = run_bass_kernel_spmd(_cache["nc"], in_maps, core_ids=list(range(NCORES)))
    outs = {
        k: np.concatenate([res.results[c][k] for c in range(NCORES)], axis=0)
        for k in ("lclean", "lsel", "probs", "mask")
    }
    return outs, res


def kernel(h, W, token_mask):
    h = np.asarray(h, dtype=np.float32)
    W = np.asarray(W, dtype=np.float32)
    tm = np.asarray(token_mask).astype(bool)

    outs, _ = _run_device(h, W)
    lclean = outs["lclean"]
    lsel = outs["lsel"]
    probs = outs["probs"]
    mask = outs["mask"].astype(bool)

    if not tm.all():
        lsel[~tm] = -np.inf
        mask[~tm] = False
        probs[~tm] = 0.0

    # Exact fixup for threshold ties (rows where ">= 8th value" selected != 8):
    bad = np.flatnonzero((mask.sum(axis=1) != K) & tm)
    for r in bad:
        order = np.argsort(-lsel[r], kind="stable")[:K]
        m = np.zeros(E, dtype=bool)
        m[order] = True
        mask[r] = m
        x = lclean[r].astype(np.float32)
        e = np.exp(x - x.max(), dtype=np.float32)
        p = (e / e.sum()).astype(np.float32)
        mp = np.where(m, p, np.float32(0.0))
        denom = np.maximum(mp.sum(), np.float32(1e-09))
        probs[r] = mp / denom

    return mask, probs, lclean, lsel


# revision 7
# speedup vs baseline: 559295.6457x; 559295.6457x over previous
"""MoE BaseRouter kernel for Trainium2 (8 NeuronCores, SPMD over tokens).

Computes, for h [T=16384, D=4096] f32, W [D, E=128] f32, token_mask [T] bool:
  logits_clean = h @ W
  logits_sel   = logits_clean + gumbel          (tau=1, temps=1)
  top-8 expert mask over logits_sel (per token)
  probs        = renormalized softmax(logits_clean) over the selected experts
returning (mask_full bool, probs f32, logits_clean f32, logits_sel f32),
mirroring the reference BaseRouter.

Strategy: tokens sharded 2048/core across 8 cores; W replicated. h is
transposed on the host (hT, [D, Tshard]) so the GEMM contraction dim D sits
on SBUF partitions. Per core: 4 groups of 512 tokens; each group accumulates
32 fp32 matmuls (W chunk stationary [128d,128E], hT chunk moving [128d,512t])
into PSUM [E,512], PE-transposes 128x128 blocks to token-major, then a short
DVE/ACT stage does top-8 (single InstMax), threshold mask, exp and masked
renormalization. Gumbel noise is a fixed PRNG constant (jax threefry,
key fold_in(key(7),1)) computed on the host CPU and streamed in.
"""

import numpy as np

T, D, E, K = 16384, 4096, 128, 8
NCORES = 8
TSH = T // NCORES  # tokens per core (2048)
NG = 4  # token groups per core
GT = TSH // NG  # tokens per group (512)
NB = GT // 128  # 128-token blocks per group (4)
NK = D // 128  # contraction chunks (32)

_cache = {}


def _apply_tile_patch(tile_mod, mybir):
    """walrus CoreV3 codegen allows at most 1 sync wait per instruction; the
    TileContext tail drain carries one wait per live proc. Spill them onto
    SP nops (order only needs every wait to precede the semaphore clear)."""
    if getattr(tile_mod.TileContext, "_drain_patched", False):
        return

    def _drain_and_barrier(self, tick_clock, wait_clock):
        nc = self.nc
        drain_inst = nc.sync.drain()
        wait_clock.add_sem_waits(
            drain_inst.ins, tile_mod.ScopedClock({None: tick_clock.global_clock})
        )
        si = drain_inst.ins.sync_info
        if si is not None and si.on_wait:
            waits = list(si.on_wait)
            del si.on_wait[:]
            for w in waits:
                nop = nc.sync.nop(nofuse=True, hint="drain_wait_spill")
                nop.ins.sync_info = mybir.SyncInfo(on_update=[], on_wait=[w])
        nc.all_engine_barrier()
        assert self.sems is not None
        popped = nc._tile_sem_poison_stack.pop()
        assert popped is self._sem_poison
        nc.clear_and_free_semaphores(list(self.sems.allocated().values()))
        nc.all_engine_barrier()

    tile_mod.TileContext._drain_and_barrier = _drain_and_barrier
    tile_mod.TileContext._drain_patched = True


def _build_nc(repeat=1):
    """Build the per-core Bass module (same program on all 8 cores).

    repeat>1 wraps the body in an on-device For_i loop (same static program
    re-executed; used only for timing measurements)."""
    from contextlib import ExitStack

    import concourse.tile as tile
    from concourse import bacc, mybir

    f32 = mybir.dt.float32
    u8 = mybir.dt.uint8

    nc = bacc.Bacc(
        "TRN2", target_bir_lowering=False, debug=False, enable_asserts=False
    )
    hT = nc.dram_tensor("hT", [D, TSH], f32, kind="ExternalInput")
    Wm = nc.dram_tensor("Wm", [D, E], f32, kind="ExternalInput")
    gum = nc.dram_tensor("gum", [TSH, E], f32, kind="ExternalInput")
    ident = nc.dram_tensor("ident", [128, 128], f32, kind="ExternalInput")
    lclean = nc.dram_tensor("lclean", [TSH, E], f32, kind="ExternalOutput")
    lsel = nc.dram_tensor("lsel", [TSH, E], f32, kind="ExternalOutput")
    probs = nc.dram_tensor("probs", [TSH, E], f32, kind="ExternalOutput")
    mask = nc.dram_tensor("mask", [TSH, E], u8, kind="ExternalOutput")

    # DRAM views: d = 128*k + p ; token = 512*g + 128*b + p
    hT_v = hT[:, :].rearrange("(k p) j -> p k j", p=128)  # [128, NK, TSH]
    W_v = Wm[:, :].rearrange("(k p) e -> p k e", p=128)  # [128, NK, E]

    def tok_view(t):  # [TSH, E] -> [NG][128, NB, E]
        return t[:, :].rearrange("(g b p) e -> g p b e", g=NG, b=NB)

    lclean_v, lsel_v, probs_v, mask_v, gum_v = (
        tok_view(x) for x in (lclean, lsel, probs, mask, gum)
    )

    with tile.TileContext(nc) as tc, ExitStack() as ctx:
        wpool = ctx.enter_context(tc.tile_pool(name="w", bufs=1))
        cpool = ctx.enter_context(tc.tile_pool(name="const", bufs=1))
        hpool = ctx.enter_context(tc.tile_pool(name="h", bufs=8))
        ppool = ctx.enter_context(tc.tile_pool(name="psum_mm", bufs=2, space="PSUM"))
        ptpool = ctx.enter_context(tc.tile_pool(name="psum_t", bufs=6, space="PSUM"))
        lgpool = ctx.enter_context(tc.tile_pool(name="lg", bufs=2))
        opool = ctx.enter_context(tc.tile_pool(name="outs", bufs=2))
        spool = ctx.enter_context(tc.tile_pool(name="small", bufs=16))

        def body():
            W_sb = wpool.tile([128, NK, E], f32)
            nc.sync.dma_start(W_sb[:], W_v)
            id_sb = cpool.tile([128, 128], f32)
            nc.sync.dma_start(id_sb[:], ident[:, :])
            _groups(W_sb, id_sb)

        def _groups(W_sb, id_sb):
          for g in range(NG):
            # ---- load hT group slab in 4 sub-DMAs of 8 chunks each ----
            hs = []
            for q in range(4):
                h_sb = hpool.tile([128, 8, GT], f32, tag="hslab")
                nc.sync.dma_start(
                    h_sb[:], hT_v[:, 8 * q : 8 * q + 8, g * GT : (g + 1) * GT]
                )
                hs.append(h_sb)

            # ---- GEMM: psum[E, GT] += W[:,k,:].T @ hT[:,k,:] over 32 chunks
            psum_g = ppool.tile([E, GT], f32, tag="psg")
            for k in range(NK):
                nc.tensor.matmul(
                    psum_g[:],
                    W_sb[:, k, :],
                    hs[k // 8][:, k % 8, :],
                    start=(k == 0),
                    stop=(k == NK - 1),
                )

            # ---- PSUM -> SBUF, then PE-transpose to token-major blocks ----
            lg_sb = lgpool.tile([E, GT], f32, tag="lg")
            nc.scalar.copy(lg_sb[:], psum_g[:])

            gum_sb = opool.tile([128, NB, E], f32, tag="gum")
            nc.sync.dma_start(gum_sb[:], gum_v[g])
            lclean_sb = opool.tile([128, NB, E], f32, tag="lclean")
            lsel_sb = opool.tile([128, NB, E], f32, tag="lsel")
            probs_sb = opool.tile([128, NB, E], f32, tag="probs")
            mask_sb = opool.tile([128, NB, E], u8, tag="mask")

            for b in range(NB):
                lt = ptpool.tile([128, E], f32, tag="pst")  # logits [tok, E] in PSUM
                nc.tensor.transpose(lt[:], lg_sb[:, 128 * b : 128 * (b + 1)], id_sb[:])

                # logits_sel = logits + gumbel (also the lsel output tile)
                xs = lsel_sb[:, b, :]
                nc.vector.tensor_tensor(xs, lt[:], gum_sb[:, b, :], mybir.AluOpType.add)

                # top-8 values; threshold = 8th largest
                m8 = spool.tile([128, 8], f32, tag="m8")
                nc.vector.max(m8[:], xs)
                thr = m8[:, 7:8]
                maskf = spool.tile([128, E], f32, tag="maskf")
                nc.vector.tensor_scalar(maskf[:], xs, thr, None, mybir.AluOpType.is_ge)
                nc.vector.tensor_scalar(
                    mask_sb[:, b, :], xs, thr, None, mybir.AluOpType.is_ge
                )

                # renormalized softmax over selected experts
                nmax = spool.tile([128, 1], f32, tag="nmax")
                nc.vector.tensor_reduce(
                    nmax[:], lt[:], mybir.AxisListType.X, mybir.AluOpType.max,
                    negate=True,
                )
                et = spool.tile([128, E], f32, tag="et")
                nc.scalar.activation(
                    et[:], lt[:], mybir.ActivationFunctionType.Exp,
                    bias=nmax[:, 0:1], scale=1.0,
                )
                nc.scalar.copy(lclean_sb[:, b, :], lt[:])
                pb = probs_sb[:, b, :]
                nc.vector.tensor_tensor(pb, et[:], maskf[:], mybir.AluOpType.mult)
                ssum = spool.tile([128, 1], f32, tag="ssum")
                nc.vector.reduce_sum(ssum[:], pb, axis=mybir.AxisListType.X)
                rec = spool.tile([128, 1], f32, tag="rec")
                nc.vector.reciprocal(rec[:], ssum[:])
                nc.vector.tensor_scalar_mul(pb, pb, rec[:, 0:1])

            nc.scalar.dma_start(lclean_v[g], lclean_sb[:])
            nc.scalar.dma_start(lsel_v[g], lsel_sb[:])
            nc.scalar.dma_start(probs_v[g], probs_sb[:])
            nc.scalar.dma_start(mask_v[g], mask_sb[:])

        if repeat == 1:
            body()
        else:
            with tc.For_i(0, repeat, 1):
                body()

    nc.compile()
    return nc


def _gumbel_np():
    """The reference's gumbel draw — a fixed constant (jax threefry on CPU)."""
    import jax
    import jax.numpy as jnp

    cpu = jax.devices("cpu")[0]
    with jax.default_device(cpu):
        kg = jax.random.fold_in(jax.random.key(7), 1)
        u = jax.random.uniform(
            kg, (T, E), minval=1e-06, maxval=1 - 1e-06, dtype=jnp.float32
        )
        g = -jnp.log(-jnp.log(u))
        return np.asarray(jax.device_get(g), dtype=np.float32)


def _run_device(h, W):
    from concourse.bass_utils import run_bass_kernel_spmd

    if "nc" not in _cache:
        _cache["nc"] = _build_nc()
    if "gum" not in _cache:
        _cache["gum"] = _gumbel_np()
    gum = _cache["gum"]
    ident = np.eye(128, dtype=np.float32)
    Wc = np.ascontiguousarray(W, dtype=np.float32)

    in_maps = []
    for c in range(NCORES):
        sl = slice(c * TSH, (c + 1) * TSH)
        in_maps.append(
            {
                "hT": np.ascontiguousarray(h[sl].T),
                "Wm": Wc,
                "gum": np.ascontiguousarray(gum[sl]),
                "ident": ident,
            }
        )
    res = run_bass_kernel_spmd(_cache["nc"], in_maps, core_ids=list(range(NCORES)))
    outs = {
        k: np.concatenate([res.results[c][k] for c in range(NCORES)], axis=0)
        for k in ("lclean", "lsel", "probs", "mask")
    }
    return outs, res


def kernel(h, W, token_mask):
    h = np.asarray(h, dtype=np.float32)
    W = np.asarray(W, dtype=np.float32)
    tm = np.asarray(token_mask).astype(bool)

    outs, _ = _run_device(h, W)
    lclean = outs["lclean"]
    lsel = outs["lsel"]
    probs = outs["probs"]
    mask = outs["mask"].astype(bool)

    if not tm.all():
        lsel[~tm] = -np.inf
        mask[~tm] = False
        probs[~tm] = 0.0

    # Exact fixup for threshold ties (rows where ">= 8th value" selected != 8):
    bad = np.flatnonzero((mask.sum(axis=1) != K) & tm)
    for r in bad:
        order = np.argsort(-lsel[r], kind="stable")[:K]
        m = np.zeros(E, dtype=bool)
        m[order] = True
        mask[r] = m
        x = lclean[r].astype(np.float32)
        e = np.exp(x - x.max(), dtype=np.float32)
        p = (e / e.sum()).astype(np.float32)
        mp = np.where(m, p, np.float32(0.0))
        denom = np.maximum(mp.sum(), np.float32(1e-09))
        probs[r] = mp / denom

    return mask, probs, lclean, lsel
